# revision 1
# baseline (speedup 1.0000x reference)
"""Trainium2 Bass kernel: batched multi-head attention softmax(Q K^T) V.

Full inputs: q/k/v [4, 16, 2048, 64] f32. Sharded over 8 NeuronCores by
flattened (batch, head): core i computes heads [8i, 8i+8).

Per-head algorithm (S=2048, D=64, P=128):
  - Q,K,V loaded bf16 (casting SWDGE DMA), Q^T/K^T built d-major via XBAR
    DMA-transposes of [128,128] bf16 tiles (striped s-order, tracked in
    index math).
  - scores^T tiles [128 t, 1024 s] on TensorE (bf16, fp32 PSUM), exp on
    ScalarE (no max subtraction: |scores| < ~50 so exp fits fp32/bf16).
  - O^T[65, 2048] = sum_t V_aug[t]^T @ E[t] accumulated in PSUM, where
    V_aug has a ones column => row 64 = softmax denominators.
  - PE-transpose O^T back to [s, d] blocks, multiply by 1/denominator on
    VectorE, DMA out fp32.
"""

import os
import sys
import numpy as np

_TRN_REPO = "/opt/trn_rl_repo"

B, H, S, D = 4, 16, 2048, 64
P = 128
N_CORES = 8
HEADS = (B * H) // N_CORES  # heads per core
TB = S // P  # 16 t-blocks

_prog_cache = {}
PHASE_MARKS = []


def _s_start(j):
    """DRAM s-offset of the 128-row block behind psum column block j.

    OT psum columns are ordered chunk-major: chunk c in [0,4) of 512 cols,
    within chunk 4 m-blocks of 128. Column (c, mm, r) holds
    s = 256*m + 128*bq + r with bq = c//2, m = 4*(c%2) + mm.
    """
    c, mm = j // 4, j % 4
    return 256 * (4 * (c % 2) + mm) + 128 * (c // 2)


def _build_program(heads=HEADS, dumps=False, reps=1, ot_lag=6, ebufs=10):
    if _TRN_REPO not in sys.path:
        sys.path.insert(0, _TRN_REPO)
    import concourse.bacc as bacc
    import concourse.mybir as mybir
    import concourse.tile as tile
    from bass_rust import add_dep_helper
    from contextlib import ExitStack

    f32 = mybir.dt.float32
    bf16 = mybir.dt.bfloat16
    EXP = mybir.ActivationFunctionType.Exp

    nc = bacc.Bacc("TRN2", target_bir_lowering=False, debug=False)
    q_d = nc.declare_dram_parameter("q", [heads, S, D], f32, isOutput=False)
    k_d = nc.declare_dram_parameter("k", [heads, S, D], f32, isOutput=False)
    v_d = nc.declare_dram_parameter("v", [heads, S, D], f32, isOutput=False)
    id_d = nc.declare_dram_parameter("ident", [P, P], f32, isOutput=False)
    o_d = nc.declare_dram_parameter("out", [heads, S, D], f32, isOutput=True)
    dump_d = {}
    if dumps:
        for nm, shape, dt_ in [
            ("qT_dump", [P, TB // 2, P], bf16),
            ("kT_dump", [P, TB // 2, P], bf16),
            ("kTs_dump", [P, TB // 2 + 1, P], bf16),
            ("e_dump", [TB, P, S], bf16),
            ("otsb_dump", [D + 1, S], f32),
        ]:
            dump_d[nm] = nc.declare_dram_parameter(nm, shape, dt_, isOutput=True)

    with tile.TileContext(nc) as tc, ExitStack() as ctx:
        pool = lambda name, bufs, **kw: ctx.enter_context(
            tc.tile_pool(name=name, bufs=bufs, **kw)
        )
        const_pool = pool("const", 1)
        qbf_pool = pool("qbf", 5)
        kbf_pool = pool("kbf", 5)
        vaug_pool = pool("vaug", 5)
        qT_pool = pool("qT", 5)
        kT_pool = pool("kT", 5)
        kTs_pool = pool("kTs", 5)
        ea_pool = pool("ea", ebufs)
        eb_pool = pool("eb", ebufs)
        otsb_pool = pool("otsb", 2)
        rec_pool = pool("rec", 8)
        obuf_pool = pool("obuf", 2)
        psA = pool("psA", 1, space="PSUM")
        psB = pool("psB", 1, space="PSUM")
        psOT = pool("psOT", 1, space="PSUM")

        ident = const_pool.tile([P, P], f32)
        warm = const_pool.tile([P, 1], f32)

        pend = {}  # head -> (q_bf, k_bf, v_aug, qT, kT)

        last_qT_tr = [None]  # previous head's last transpose (pacing anchor)

        def issue_loads(hd):
            PHASE_MARKS.append((nc.next_id(), f"loads_h{hd}"))
            # K first (its transposes gate the first score matmul), V last.
            # Every DMA-transpose conservatively waits ALL previously issued
            # DMA copies (xbar hazard serialization), so pace copy issue:
            # loads of head h+1 and this head's V load go behind this head's
            # transposes via explicit dep edges.
            # K staging padded by one 64-col zero block on each side so the
            # shifted transposes below can cover edge t-blocks.
            k_bf = kbf_pool.tile([P, TB + 2, D], bf16)
            nc.vector.memset(k_bf[:, 0, :], 0.0)
            nc.vector.memset(k_bf[:, TB + 1, :], 0.0)
            k_ld = nc.gpsimd.dma_start(
                out=k_bf[:, 1 : TB + 1, :],
                in_=k_d[hd % heads].rearrange("(n p) d -> p n d", p=P),
            )
            q_bf = qbf_pool.tile([P, TB, D], bf16)
            q_ld = nc.gpsimd.dma_start(
                out=q_bf[:], in_=q_d[hd % heads].rearrange("(n p) d -> p n d", p=P)
            )
            if last_qT_tr[0] is not None:
                add_dep_helper(k_ld.ins, last_qT_tr[0], reason="pace loads behind prev transposes")
                add_dep_helper(q_ld.ins, last_qT_tr[0], reason="pace loads behind prev transposes")
            # Natural transposes: slot m has t-block 2m on partitions 0-63 and
            # t-block 2m+1 on partitions 64-127 (k_bf block i holds t-block i-1).
            kT = kT_pool.tile([P, TB // 2, P], bf16)
            nc.sync.dma_start(out=kT[:], in_=k_bf[:, 1 : TB + 1, :], transpose=True)
            # Shifted transposes: slot m has t-block 2m-1 on partitions 0-63
            # and t-block 2m on partitions 64-127 (junk/zero at the edges).
            kTs = kTs_pool.tile([P, TB // 2 + 1, P], bf16)
            nc.sync.dma_start(out=kTs[:], in_=k_bf[:, 0 : TB + 2, :], transpose=True)
            # Batched xbar transposes: out[:, m, :] = in[:, 128m:128(m+1)].T
            qT = qT_pool.tile([P, TB // 2, P], bf16)
            qT_tr = nc.sync.dma_start(out=qT[:], in_=q_bf[:], transpose=True)
            v_aug = vaug_pool.tile([P, TB, D + 1], bf16)
            nc.vector.memset(v_aug[:], 1.0)
            v_ld = nc.gpsimd.dma_start(
                out=v_aug[:, :, 0:D], in_=v_d[hd % heads].rearrange("(n p) d -> p n d", p=P)
            )
            add_dep_helper(v_ld.ins, qT_tr.ins, reason="v load after transposes")
            last_qT_tr[0] = qT_tr.ins
            if dumps and hd == 0:
                nc.sync.dma_start(out=dump_d["qT_dump"][:], in_=qT[:])
                nc.sync.dma_start(out=dump_d["kT_dump"][:], in_=kT[:])
                nc.sync.dma_start(out=dump_d["kTs_dump"][:], in_=kTs[:])
            pend[hd] = (q_bf, k_bf, v_aug, qT, kT, kTs)

        def kt_block(kT, kTs, tb, bq):
            """lhsT [64, 128] for t-block tb based at partition 64*bq."""
            lo = 64 * bq
            if bq == tb % 2:
                return kT[lo : lo + 64, tb // 2, :]
            if bq == 0:  # tb odd: shifted slot (tb+1)//2, lower half
                return kTs[0:64, (tb + 1) // 2, :]
            return kTs[64:128, tb // 2, :]  # tb even, upper half

        OT_LAG = ot_lag
        done = {}  # head -> ot_sb awaiting finish

        def emit_ot(ot, v_aug, e_tiles, tb):
            vt = v_aug[:, tb, :]
            e_a, e_b = e_tiles.pop(tb)
            for c in range(4):
                e_src = e_a if c < 2 else e_b
                nc.tensor.matmul(
                    ot[:, 512 * c : 512 * (c + 1)],
                    lhsT=vt,
                    rhs=e_src[:, 512 * (c % 2) : 512 * (c % 2 + 1)],
                    start=(tb == 0),
                    stop=(tb == TB - 1),
                )

        def finish_head(hd, last=False):
            PHASE_MARKS.append((nc.next_id(), f"finish_h{hd}"))
            ot_sb = done.pop(hd)
            tr = psOT.tile([P, TB, P], f32, tag="otslot")
            obuf = obuf_pool.tile([P, 8, 2, D], f32)
            for j in range(TB):
                nc.tensor.transpose(
                    tr[:, j, 0 : D + 1],
                    ot_sb[:, P * j : P * (j + 1)],
                    ident[0 : D + 1, 0 : D + 1],
                )
            dst = o_d[hd % heads].rearrange("(m b p) d -> p m b d", m=8, b=2, p=P)
            for j in range(TB):
                rec = rec_pool.tile([P, 1], f32)
                nc.vector.reciprocal(rec[:], tr[:, j, D : D + 1])
                nc.vector.tensor_scalar_mul(
                    obuf[:, j % 8, j // 8, :], tr[:, j, 0:D], rec[:]
                )
                if last and j == 7:
                    # tail only: first half stores while second half normalizes
                    nc.sync.dma_start(out=dst[:, :, 0, :], in_=obuf[:, :, 0, :])
            if last:
                nc.sync.dma_start(out=dst[:, :, 1, :], in_=obuf[:, :, 1, :])
            else:
                # block j covers DRAM rows s = 256*(j%8) + 128*(j//8) + p: one
                # strided store for the whole head.
                nc.sync.dma_start(out=dst[:], in_=obuf[:])

        # Global software-pipelined stream over (head, t-block) units.
        # Scores+exp for unit g are emitted at step g; the OT accumulation for
        # unit g-OT_LAG follows, so the OT tail of head h interleaves with the
        # first score blocks of head h+1 and ACT never waits on it.
        heads_ctx = {}  # head -> (v_aug, e_tiles, ot)
        PREFETCH = 3
        total = heads * reps

        def emit_unit(g):
            hd, tb = divmod(g, TB)
            if tb == 0:
                PHASE_MARKS.append((nc.next_id(), f"score_h{hd}"))
                _q_bf, _k_bf, v_aug, qT, kT, kTs = pend.pop(hd)
                heads_ctx[hd] = {"v": v_aug, "qT": qT, "kT": kT, "kTs": kTs,
                                 "e": {}, "ot": None}
            ctx_h = heads_ctx[hd]
            qT, kT, kTs = ctx_h["qT"], ctx_h["kT"], ctx_h["kTs"]
            stA = psA.tile([P, 1024], f32)
            for g2 in range(2):
                nc.tensor.matmul(
                    stA[:, 512 * g2 : 512 * (g2 + 1)],
                    lhsT=kt_block(kT, kTs, tb, 0),
                    rhs=qT[0:64, 4 * g2 : 4 * g2 + 4, :],
                    start=True,
                    stop=True,
                )
            e_a = ea_pool.tile([P, 1024], bf16)
            nc.scalar.activation(e_a[:], stA[:], EXP)
            stB = psB.tile([P, 1024], f32)
            for g2 in range(2):
                nc.tensor.matmul(
                    stB[:, 512 * g2 : 512 * (g2 + 1)],
                    lhsT=kt_block(kT, kTs, tb, 1),
                    rhs=qT[64:128, 4 * g2 : 4 * g2 + 4, :],
                    start=True,
                    stop=True,
                )
            e_b = eb_pool.tile([P, 1024], bf16)
            nc.scalar.activation(e_b[:], stB[:], EXP)
            if dumps and hd == 0:
                nc.sync.dma_start(out=dump_d["e_dump"][tb, :, 0:1024], in_=e_a[:])
                nc.sync.dma_start(out=dump_d["e_dump"][tb, :, 1024:2048], in_=e_b[:])
            ctx_h["e"][tb] = (e_a, e_b)
            if tb == OT_LAG and hd >= 1:
                finish_head(hd - 1)

        def emit_ot_unit(g):
            hd, tb = divmod(g, TB)
            ctx_h = heads_ctx[hd]
            if tb == 0:
                ot_tile = psOT.tile([D + 1, S], f32, tag="otslot")
                ctx_h["ot"] = ot_tile
            emit_ot(ctx_h["ot"], ctx_h["v"], ctx_h["e"], tb)
            if tb == TB - 1:
                ot_sb = otsb_pool.tile([D + 1, S], f32)
                nc.vector.tensor_copy(ot_sb[:], ctx_h["ot"][:])
                if dumps and hd == 0:
                    nc.sync.dma_start(out=dump_d["otsb_dump"][:], in_=ot_sb[:])
                done[hd] = ot_sb
                heads_ctx.pop(hd)

        n_units = total * TB
        for g in range(n_units + OT_LAG):
            gh = g // TB
            if g == 0:
                for pf in range(min(PREFETCH, total)):
                    issue_loads(pf)
                # warm the exp table set + load the transpose identity while
                # the first input DMAs are in flight
                nc.vector.memset(warm[:], 0.0)
                nc.scalar.activation(warm[:], warm[:], EXP)
                nc.sync.dma_start(out=ident[:], in_=id_d[:])
            elif g % TB == 0 and gh + PREFETCH - 1 < total:
                issue_loads(gh + PREFETCH - 1)
            if g < n_units:
                emit_unit(g)
            if g >= OT_LAG:
                emit_ot_unit(g - OT_LAG)
        finish_head(total - 1, last=True)

    PHASE_MARKS.append((nc.next_id(), "END"))
    if not nc.is_finalized():
        nc.finalize()
    return nc


def _get_program():
    if "nc" not in _prog_cache:
        _prog_cache["nc"] = _build_program()
    return _prog_cache["nc"]


def _run(in_maps, trace=False):
    if _TRN_REPO not in sys.path:
        sys.path.insert(0, _TRN_REPO)
    from concourse.bass_utils import run_bass_kernel_spmd

    nc = _get_program()
    return run_bass_kernel_spmd(nc, in_maps, list(range(N_CORES)), trace=trace)


def _make_in_maps(input_query, input_key, input_value):
    q = np.ascontiguousarray(np.asarray(input_query, np.float32)).reshape(B * H, S, D)
    k = np.ascontiguousarray(np.asarray(input_key, np.float32)).reshape(B * H, S, D)
    v = np.ascontiguousarray(np.asarray(input_value, np.float32)).reshape(B * H, S, D)
    ident = np.eye(P, dtype=np.float32)
    in_maps = []
    for i in range(N_CORES):
        sl = slice(i * HEADS, (i + 1) * HEADS)
        in_maps.append(
            {
                "q": np.ascontiguousarray(q[sl]),
                "k": np.ascontiguousarray(k[sl]),
                "v": np.ascontiguousarray(v[sl]),
                "ident": ident,
            }
        )
    return in_maps


def kernel(input_query, input_key, input_value):
    in_maps = _make_in_maps(input_query, input_key, input_value)
    res = _run(in_maps, trace=False)
    out = np.concatenate([np.asarray(r["out"]) for r in res.results], axis=0)
    return out.reshape(B, H, S, D).astype(np.float32)


def kernel_traced(input_query, input_key, input_value):
    """Like kernel() but with neuron-profile tracing; returns (out, results)."""
    in_maps = _make_in_maps(input_query, input_key, input_value)
    res = _run(in_maps, trace=True)
    out = np.concatenate([np.asarray(r["out"]) for r in res.results], axis=0)
    return out.reshape(B, H, S, D).astype(np.float32), res

